# revision 9
# baseline (speedup 1.0000x reference)
"""Trainium2 Bass kernel for nn_MHA_63118839382398.

Full MHA block: fused QKV projection, per-head RMSNorm on q/k, rotate-half
RoPE, causal softmax attention, output projection.

Sharding over 8 NeuronCores: core c handles batch b = c//2 and heads
[8*(c%2), 8*(c%2)+8) (tensor parallel over head halves within a batch
pair). Each core computes a partial out-projection; a 2-rank
ReduceScatter over each pair sums the partials and leaves each core with
half of that batch's token rows, which the host reassembles.

Layout strategy (all transposed, feats x tokens), so every matmul
contraction sits on the partition axis with no on-chip transposes except
V (cheap PE-mode 128x128 transposes):
  - xT (dmodel, ntok) per batch, host-pretransposed.
  - qT/kT = W @ xT  -> (head_dim, ntok), float32r matmuls.
  - scores computed transposed: S^T = K @ Q^T (keys part, queries free),
    bf16; exp on ACT; softmax denominator via a ones-column appended to
    the V stationary operand of the P@V matmul; 1/sum applied at the end.
  - RMS factors are broadcast with gpsimd partition_broadcast; the
    qn_w/kn_w gains are folded into the host-built RoPE tables.
"""

import sys

if "/opt/trn_rl_repo" not in sys.path:
    sys.path.insert(0, "/opt/trn_rl_repo")

import numpy as np
import ml_dtypes

import concourse.bass as bass
import concourse.tile as tile
from concourse import bacc, mybir
from concourse.bass_utils import run_bass_kernel_spmd
from concourse.masks import make_identity

# Problem constants (hardcoded per harness contract).
B = 4
N = 2048
D_MODEL = 1024
N_HEADS = 16
D_HEAD = 64
ROPE_BASE = 10000.0
EPS = float(np.finfo(np.float32).eps)
N_CORES = 8

HPC = N_HEADS // 2          # heads per core = 8
WAVES = HPC // 2            # head-pair waves = 4
TOKCH = 512                 # token chunk for projections / q chunks
NT = N // TOKCH             # 4
QT = 128                    # query tile for mask classification
NQT = N // QT               # 16
KB = 128                    # key block
NKB = N // KB               # 16
DC = 128                    # dmodel chunk
NDC = D_MODEL // DC         # 8

F32 = mybir.dt.float32
F32R = mybir.dt.float32r
BF16 = mybir.dt.bfloat16
BF = ml_dtypes.bfloat16

_CACHE = {}


def _classify_mask(mask):
    """Per (key-block, query-tile) classification of the additive mask.

    Returns (state[NKB][NQT], patterns) where state is 'skip' (all
    masked), 'full' (none masked), or an integer index into patterns:
    a list of unique (128,128) float bf16 0/1 tiles indexed [key, query].
    """
    mask = np.asarray(mask)
    assert mask.shape == (N, N)
    patterns = []
    pat_keys = {}
    state = [[None] * NQT for _ in range(NKB)]
    for kb in range(NKB):
        for qt in range(NQT):
            blk = mask[qt * QT : (qt + 1) * QT, kb * KB : (kb + 1) * KB]
            if blk.all():
                state[kb][qt] = "skip"
            elif not blk.any():
                state[kb][qt] = "full"
            else:
                # tile indexed [key, query] = transpose of mask block
                tileq = (~blk.T).astype(BF)
                key = tileq.tobytes()
                if key not in pat_keys:
                    pat_keys[key] = len(patterns)
                    patterns.append(tileq)
                state[kb][qt] = pat_keys[key]
    return state, patterns


def _build_program(state, n_patterns):
    """Build the SPMD Bass program (same graph on all 8 cores)."""
    nc = bacc.Bacc(
        "TRN2", target_bir_lowering=False, debug=False, num_devices=N_CORES
    )

    p_xt = nc.dram_tensor("xt", [D_MODEL, N], F32R, kind="ExternalInput").ap()
    p_wqk = nc.dram_tensor("wqk", [D_MODEL, 1024], F32R, kind="ExternalInput").ap()
    p_wv = nc.dram_tensor("wv", [D_MODEL, 512], F32R, kind="ExternalInput").ap()
    p_wo = nc.dram_tensor("wo", [512, D_MODEL], BF16, kind="ExternalInput").ap()
    # rope tables with qn_w/kn_w folded: rows 0..127 (4x32 freq pattern)
    p_rope = nc.dram_tensor("rope", [128, 4, N], BF16, kind="ExternalInput").ap()
    p_wfold = nc.dram_tensor("wfold", [2, 128], F32, kind="ExternalInput").ap()
    p_pswap = nc.dram_tensor("pswap", [128, 128], BF16, kind="ExternalInput").ap()
    p_ind2 = nc.dram_tensor("ind2", [128, 2], F32R, kind="ExternalInput").ap()
    if n_patterns:
        p_pat = nc.dram_tensor(
            "pat", [128, n_patterns, 128], BF16, kind="ExternalInput"
        ).ap()
    p_out = nc.dram_tensor("out", [N // 2, D_MODEL], F32, kind="ExternalOutput").ap()

    y_part = nc.dram_tensor("y_part", [N, D_MODEL], F32)
    rs_out = nc.dram_tensor("rs_out", [N // 2, D_MODEL], F32)

    # per (kb, qchunk): first query tile (within chunk) not fully masked,
    # and count of key blocks participating per qchunk
    QPC = TOKCH // QT  # query tiles per chunk = 4
    n_kb = [0] * NT
    qlo_t = {}
    for qc in range(NT):
        for kb in range(NKB):
            sub = [state[kb][qc * QPC + j] for j in range(QPC)]
            if all(s == "skip" for s in sub):
                continue
            n_kb[qc] = max(n_kb[qc], kb + 1)
            lead = 0
            while sub[lead] == "skip":
                lead += 1
            qlo_t[(qc, kb)] = lead

    with tile.TileContext(nc) as tc:
        import contextlib

        ctx = contextlib.ExitStack()
        with ctx:
            singles = ctx.enter_context(tc.tile_pool(name="singles", bufs=1))
            wpool = ctx.enter_context(tc.tile_pool(name="wpool", bufs=1))
            wavep = ctx.enter_context(tc.tile_pool(name="wavep", bufs=2))
            work = ctx.enter_context(tc.tile_pool(name="work", bufs=2))
            espool = ctx.enter_context(tc.tile_pool(name="es", bufs=3))
            epi = ctx.enter_context(tc.tile_pool(name="epi", bufs=2))
            outp = ctx.enter_context(tc.tile_pool(name="outp", bufs=2))

            pp = ctx.enter_context(tc.tile_pool(name="pp", bufs=2, space="PSUM"))
            ps = ctx.enter_context(tc.tile_pool(name="ps", bufs=2, space="PSUM"))
            pv = ctx.enter_context(tc.tile_pool(name="pv", bufs=2, space="PSUM"))
            pss = ctx.enter_context(tc.tile_pool(name="pss", bufs=2, space="PSUM"))

            # ---- resident constants -------------------------------------
            xt_sb = singles.tile([128, NDC, N], F32R)  # 64KB/part
            for dc in range(NDC):
                nc.sync.dma_start(
                    out=xt_sb[:, dc, :], in_=p_xt[dc * DC : (dc + 1) * DC, :]
                )
            rope_sb = singles.tile([128, 4, N], BF16)  # cq, sq, ck, sk
            nc.sync.dma_start(out=rope_sb, in_=p_rope)
            ident = singles.tile([128, 128], BF16)
            make_identity(nc, ident)
            ones_col = singles.tile([128, 64], F32)
            nc.vector.memset(ones_col, 1.0)
            wfold = singles.tile([2, 128], F32)
            nc.sync.dma_start(out=wfold, in_=p_wfold)
            pswap = singles.tile([128, 128], BF16)
            nc.sync.dma_start(out=pswap, in_=p_pswap)
            eps_sb = singles.tile([128, 1], F32)
            nc.vector.memset(eps_sb, EPS)
            ind2 = singles.tile([128, 2], F32R)
            nc.sync.dma_start(out=ind2, in_=p_ind2)
            if n_patterns:
                pat_sb = singles.tile([128, n_patterns, 128], BF16)
                nc.sync.dma_start(out=pat_sb, in_=p_pat)
            yt_sb = singles.tile([128, WAVES, N], BF16)  # 8KB/part

            def proj_chunk(dst_ps, w_sb, wcol, t, nw=8):
                """dst_ps[[128,TOKCH]] += w[:, wcol:wcol+128].T @ xT[:, chunk t]"""
                for dc in range(NDC):
                    nc.tensor.matmul(
                        dst_ps,
                        lhsT=w_sb[:, dc, wcol : wcol + 128],
                        rhs=xt_sb[:, dc, t * TOKCH : (t + 1) * TOKCH],
                        start=(dc == 0),
                        stop=(dc == nw - 1),
                    )

            def rms_rope_chunk(raw_ps, t, rope_idx, qk_rot, launch_sq=True):
                """raw (128, TOKCH) f32 psum -> normed+roped bf16 into
                qk_rot[:, t*TOKCH:...]. rope_idx: 0 for q tables, 1 for k."""
                tsl = slice(t * TOKCH, (t + 1) * TOKCH)
                raw = work.tile([128, TOKCH], F32, tag="raw")
                nc.scalar.activation(raw, raw_ps, mybir.ActivationFunctionType.Copy)
                sq = work.tile([128, TOKCH], F32R, tag="sq")
                nc.scalar.activation(sq, raw_ps, mybir.ActivationFunctionType.Square)
                ss = pss.tile([2, TOKCH], F32, tag="small", name="ss")
                nc.tensor.matmul(
                    ss, lhsT=ind2, rhs=sq,
                    start=True, stop=True,
                )
                lnm = work.tile([2, TOKCH], F32, tag="lnm")
                nc.scalar.activation(
                    lnm, ss, mybir.ActivationFunctionType.Ln,
                    bias=eps_sb[0:2, :], scale=1.0 / D_HEAD,
                )
                inv = work.tile([2, TOKCH], F32, tag="inv")
                nc.scalar.activation(
                    inv, lnm, mybir.ActivationFunctionType.Exp, scale=-0.5
                )
                fac = pss.tile([128, TOKCH], F32, tag="small", name="fac")
                nc.tensor.matmul(fac, lhsT=wfold, rhs=inv, start=True, stop=True)
                qn = work.tile([128, TOKCH], BF16, tag="qn")
                nc.vector.tensor_mul(qn, raw, fac)
                # rope: rot = qn * cos_tbl + swap(qn) * sin_tbl
                cos_t = rope_sb[:, 2 * rope_idx, tsl]
                sin_t = rope_sb[:, 2 * rope_idx + 1, tsl]
                qcos = work.tile([128, TOKCH], BF16, tag="qcos")
                nc.vector.tensor_mul(qcos, qn, cos_t)
                swap_ps = pss.tile([128, TOKCH], F32, tag="small", name="swap_ps")
                nc.tensor.matmul(swap_ps, lhsT=pswap, rhs=qn, start=True, stop=True)
                qsin = work.tile([128, TOKCH], BF16, tag="qsin")
                nc.vector.tensor_mul(qsin, swap_ps, sin_t)
                nc.vector.tensor_add(qk_rot[:, tsl], qcos, qsin)

            for w in range(WAVES):
                # ---- load this wave's weight slices ---------------------
                wq_sb = wpool.tile([128, NDC, 128], F32R, tag="wq")
                nc.sync.dma_start(
                    out=wq_sb,
                    in_=p_wqk[:, w * 128 : (w + 1) * 128].rearrange(
                        "(c p) f -> p c f", p=128
                    ),
                )
                wk_sb = wpool.tile([128, NDC, 128], F32R, tag="wk")
                nc.sync.dma_start(
                    out=wk_sb,
                    in_=p_wqk[:, 512 + w * 128 : 512 + (w + 1) * 128].rearrange(
                        "(c p) f -> p c f", p=128
                    ),
                )
                wv_sb = wpool.tile([128, NDC, 128], F32R, tag="wv")
                nc.sync.dma_start(
                    out=wv_sb,
                    in_=p_wv[:, w * 128 : (w + 1) * 128].rearrange(
                        "(c p) f -> p c f", p=128
                    ),
                )

                q_rot = wavep.tile([128, N], BF16, tag="qrot")
                k_rot = wavep.tile([128, N], BF16, tag="krot")
                # V per wave: 16 key blocks x (128 tok, [v_h 64 | 1 | v_h' 64 | 1])
                v_sb = wavep.tile([128, NKB, 130], BF16, tag="v")
                nc.vector.memset(v_sb[:, :, 64:65], 1.0)
                nc.vector.memset(v_sb[:, :, 129:130], 1.0)

                for t in range(NT):
                    pq = pp.tile([128, TOKCH], F32, tag="proj")
                    proj_chunk(pq, wq_sb, 0, t)
                    rms_rope_chunk(pq, t, 0, q_rot)
                    pk = pp.tile([128, TOKCH], F32, tag="proj")
                    proj_chunk(pk, wk_sb, 0, t)
                    rms_rope_chunk(pk, t, 1, k_rot)
                    pvv = pp.tile([128, TOKCH], F32, tag="proj")
                    proj_chunk(pvv, wv_sb, 0, t)
                    vt = work.tile([128, TOKCH], BF16, tag="vt")
                    nc.scalar.activation(
                        vt, pvv, mybir.ActivationFunctionType.Copy
                    )
                    # transpose 128x128 sub-blocks: (vfeat, tok) -> (tok, vfeat)
                    for s in range(TOKCH // 128):
                        kb = t * (TOKCH // 128) + s
                        ptr = pp.tile([128, 128], BF16, tag="proj")
                        nc.tensor.transpose(
                            ptr, vt[:, s * 128 : (s + 1) * 128], ident
                        )
                        nc.vector.tensor_copy(v_sb[:, kb, 0:64], ptr[:, 0:64])
                        nc.vector.tensor_copy(v_sb[:, kb, 65:129], ptr[:, 64:128])

                # ---- attention for this wave ----------------------------
                for qc in range(NT):
                    po = [
                        pv.tile([65, TOKCH], F32, tag="pv", name=f"po{h2}")
                        for h2 in range(2)
                    ]
                    first_kb = [True, True]
                    for kb in range(n_kb[qc]):
                        if (qc, kb) not in qlo_t:
                            continue
                        qlo = qlo_t[(qc, kb)] * QT
                        csl = slice(qc * TOKCH + qlo, (qc + 1) * TOKCH)
                        osl = slice(qlo, TOKCH)
                        last = kb == n_kb[qc] - 1
                        es = [None, None]
                        for h2 in range(2):
                            hr = slice(64 * h2, 64 * h2 + 64)
                            pst = ps.tile([128, TOKCH], F32, tag="s")
                            nc.tensor.matmul(
                                pst[:, osl],
                                lhsT=k_rot[hr, kb * KB : (kb + 1) * KB],
                                rhs=q_rot[hr, csl],
                                start=True,
                                stop=True,
                            )
                            e = espool.tile([128, TOKCH], BF16, tag="es")
                            nc.scalar.activation(
                                e[:, osl],
                                pst[:, osl],
                                mybir.ActivationFunctionType.Exp,
                                scale=float(D_HEAD) ** -0.5,
                            )
                            es[h2] = e
                        # apply partial-block 0/1 masks (shared by heads)
                        for j in range(qlo // QT, QPC):
                            st = state[kb][qc * QPC + j]
                            if isinstance(st, int):
                                jsl = slice(j * QT, (j + 1) * QT)
                                for h2 in range(2):
                                    nc.vector.tensor_mul(
                                        es[h2][:, jsl],
                                        es[h2][:, jsl],
                                        pat_sb[:, st, :],
                                    )
                        for h2 in range(2):
                            nc.tensor.matmul(
                                po[h2][:, osl],
                                lhsT=v_sb[:, kb, 65 * h2 : 65 * h2 + 65],
                                rhs=es[h2][:, osl],
                                start=first_kb[h2],
                                stop=last,
                            )
                            first_kb[h2] = False
                    # normalize and store yT
                    for h2 in range(2):
                        rec_t = epi.tile([65, TOKCH], F32, tag="recip")
                        nc.vector.reciprocal(rec_t[64:65, :], po[h2][64:65, :])
                        f2 = pss.tile([64, TOKCH], F32, tag="small", name="f2")
                        nc.tensor.matmul(
                            f2,
                            lhsT=ones_col[64:65, :],
                            rhs=rec_t[64:65, :],
                            start=True,
                            stop=True,
                        )
                        yraw = epi.tile([64, TOKCH], F32, tag="yraw")
                        nc.vector.tensor_copy(yraw, po[h2][0:64, :])
                        nc.vector.tensor_mul(
                            yt_sb[
                                64 * h2 : 64 * h2 + 64,
                                w,
                                qc * TOKCH : (qc + 1) * TOKCH,
                            ],
                            yraw,
                            f2,
                        )

            # ---- output projection (bf16) -------------------------------
            wo_sb = singles.tile([128, 4, D_MODEL], BF16)
            nc.sync.dma_start(
                out=wo_sb, in_=p_wo.rearrange("(c p) f -> p c f", p=128)
            )
            for t2 in range(NQT):
                for ec in range(2):
                    pot = pp.tile([128, TOKCH], F32, tag="proj")
                    for fc in range(4):
                        nc.tensor.matmul(
                            pot,
                            lhsT=yt_sb[:, fc, t2 * 128 : (t2 + 1) * 128],
                            rhs=wo_sb[:, fc, ec * TOKCH : (ec + 1) * TOKCH],
                            start=(fc == 0),
                            stop=(fc == 3),
                        )
                    osb = outp.tile([128, TOKCH], F32, tag="o")
                    nc.scalar.activation(
                        osb, pot, mybir.ActivationFunctionType.Copy
                    )
                    nc.sync.dma_start(
                        out=y_part.ap()[
                            t2 * 128 : (t2 + 1) * 128,
                            ec * TOKCH : (ec + 1) * TOKCH,
                        ],
                        in_=osb,
                    )

            nc.gpsimd.collective_compute(
                "ReduceScatter",
                mybir.AluOpType.add,
                ins=[y_part.ap().opt()],
                outs=[rs_out.ap().opt()],
                replica_groups=[[0, 1], [2, 3], [4, 5], [6, 7]],
            )
            nc.sync.dma_start(out=p_out, in_=rs_out.ap())

    nc.compile()
    return nc


def _host_prep(x, mask, pos, W_qkv, W_out, qn_w, kn_w):
    x = np.asarray(x, dtype=np.float32)
    mask = np.asarray(mask)
    pos = np.asarray(pos).astype(np.float64)
    W_qkv = np.asarray(W_qkv, dtype=np.float32)
    W_out = np.asarray(W_out, dtype=np.float32)
    qn_w = np.asarray(qn_w, dtype=np.float32)
    kn_w = np.asarray(kn_w, dtype=np.float32)

    # rope tables (32, N)
    inv_freq = 1.0 / (ROPE_BASE ** (np.arange(0, D_HEAD, 2, dtype=np.float64) / D_HEAD))
    ang = pos[:, None] * inv_freq[None, :]  # (N, 32)
    cosT = np.cos(ang).T.astype(np.float32)  # (32, N)
    sinT = np.sin(ang).T.astype(np.float32)

    def rope_tables(w):
        # cos table row p: cos[p%32] * w[p%64]
        cos_d = np.tile(cosT, (4, 1)) * np.tile(w, 2)[:, None]
        # sin table row p, j=p%64: j<32: -sin[j]*w[j+32]; else +sin[j-32]*w[j-32]
        sin_half = np.concatenate(
            [-sinT * w[32:64][:, None], sinT * w[0:32][:, None]], axis=0
        )  # (64, N)
        sin_d = np.tile(sin_half, (2, 1))
        return cos_d, sin_d

    cq, sq = rope_tables(qn_w)
    ck, sk = rope_tables(kn_w)
    rope = np.stack([cq, sq, ck, sk], axis=1).astype(BF)  # (128, 4, N)

    pswap_np = np.zeros((128, 128), dtype=np.float32)
    for a in range(2):
        for r in range(32):
            pswap_np[64 * a + r, 64 * a + 32 + r] = 1.0
            pswap_np[64 * a + 32 + r, 64 * a + r] = 1.0
    pswap_np = pswap_np.astype(BF)

    wfold_np = np.zeros((2, 128), dtype=np.float32)
    wfold_np[0, 0:64] = 1.0
    wfold_np[1, 64:128] = 1.0
    ind2_np = np.ascontiguousarray(wfold_np.T)

    state, patterns = _classify_mask(mask)
    if patterns:
        pat = np.stack(patterns, axis=1).astype(BF)  # (128, n, 128)
    else:
        pat = None

    # per-head row indices in W_qkv
    q_rows = lambda h: slice(h * 192, h * 192 + 64)
    k_rows = lambda h: slice(h * 192 + 64, h * 192 + 128)
    v_rows = lambda h: slice(h * 192 + 128, h * 192 + 192)

    in_maps = []
    for c in range(N_CORES):
        b, half = divmod(c, 2)
        hs = [8 * half + i for i in range(8)]
        wqk = np.concatenate(
            [W_qkv[q_rows(h)] for h in hs] + [W_qkv[k_rows(h)] for h in hs], axis=0
        ).T  # (1024, 1024)
        wv = np.concatenate([W_qkv[v_rows(h)] for h in hs], axis=0).T  # (1024, 512)
        wo = W_out[:, 512 * half : 512 * half + 512].T  # (512, 1024)
        m = {
            "xt": np.ascontiguousarray(x[b].T),
            "wfold": wfold_np,
            "pswap": pswap_np,
            "ind2": ind2_np,
            "wqk": np.ascontiguousarray(wqk),
            "wv": np.ascontiguousarray(wv),
            "wo": np.ascontiguousarray(wo).astype(BF),
            "rope": rope,
        }
        if pat is not None:
            m["pat"] = pat
        in_maps.append(m)
    return in_maps, state, (0 if pat is None else pat.shape[1])


def kernel(x, mask, pos, W_qkv, W_out, qn_w, kn_w, _trace=False):
    in_maps, state, n_pat = _host_prep(x, mask, pos, W_qkv, W_out, qn_w, kn_w)
    key = (str(state), n_pat)
    if key not in _CACHE:
        _CACHE[key] = _build_program(state, n_pat)
    nc = _CACHE[key]
    res = run_bass_kernel_spmd(nc, in_maps, list(range(N_CORES)), trace=_trace)
    out = np.empty((B, N, D_MODEL), dtype=np.float32)
    for b in range(B):
        out[b, : N // 2] = res.results[2 * b]["out"]
        out[b, N // 2 :] = res.results[2 * b + 1]["out"]
    kernel._last_results = res
    return out


# revision 11
# speedup vs baseline: 1.1758x; 1.1758x over previous
"""Trainium2 Bass kernel for nn_MHA_63118839382398.

Full MHA block: fused QKV projection, per-head RMSNorm on q/k, rotate-half
RoPE, causal softmax attention, output projection.

Sharding over 8 NeuronCores: core c handles batch b = c//2 and heads
[8*(c%2), 8*(c%2)+8) (tensor parallel over head halves within a batch
pair). Each core computes a partial out-projection; a 2-rank
ReduceScatter (bf16) over each pair sums the partials and leaves each
core with half of that batch's token rows, which the host reassembles.

Layout strategy (all transposed, feats x tokens), so every matmul
contraction sits on the partition axis with no on-chip transposes except
V (cheap PE-mode 128x128 transposes):
  - xT (dmodel, ntok) per batch, host-pretransposed, bf16.
  - qT/kT = W @ xT  -> (head_dim, ntok) psum f32.
  - RMS factors: sumsq via indicator matmul, batched Ln/Exp per wave
    (one ACT table set switch pair per wave), broadcast over the 128
    rows via a tiny indicator matmul.
  - RoPE rotate-half swap via a PE permutation matmul; the qn_w/kn_w
    gains are folded into the host-built cos/sin tables.
  - scores computed transposed: S^T = K @ Q^T (keys part, queries free),
    bf16, causal blocks only; exp on ACT with the 1/sqrt(d) scale folded
    in; softmax denominator via a ones-column appended to the V
    stationary operand of the P@V matmul; 1/sum applied at the end
    (bf16 DVE reciprocal, broadcast via ones-matmul).
"""

import sys

if "/opt/trn_rl_repo" not in sys.path:
    sys.path.insert(0, "/opt/trn_rl_repo")

import numpy as np
import ml_dtypes

import concourse.bass as bass
import concourse.tile as tile
from concourse import bacc, mybir
from concourse.bass_utils import run_bass_kernel_spmd
from concourse.masks import make_identity

# Problem constants (hardcoded per harness contract).
B = 4
N = 2048
D_MODEL = 1024
N_HEADS = 16
D_HEAD = 64
ROPE_BASE = 10000.0
EPS = float(np.finfo(np.float32).eps)
N_CORES = 8

HPC = N_HEADS // 2          # heads per core = 8
WAVES = HPC // 2            # head-pair waves = 4
TOKCH = 512                 # token chunk for projections / q chunks
NT = N // TOKCH             # 4
QT = 128                    # query tile for mask classification
NQT = N // QT               # 16
KB = 128                    # key block
NKB = N // KB               # 16
DC = 128                    # dmodel chunk
NDC = D_MODEL // DC         # 8

F32 = mybir.dt.float32
BF16 = mybir.dt.bfloat16
BF = ml_dtypes.bfloat16

ACT = mybir.ActivationFunctionType

_CACHE = {}


def _classify_mask(mask):
    """Per (key-block, query-tile) classification of the mask.

    Returns (state[NKB][NQT], patterns): state is 'skip' (all masked),
    'full' (none masked), or an index into patterns: unique (128,128)
    bf16 0/1 tiles indexed [key, query]."""
    mask = np.asarray(mask)
    assert mask.shape == (N, N)
    patterns = []
    pat_keys = {}
    state = [[None] * NQT for _ in range(NKB)]
    for kb in range(NKB):
        for qt in range(NQT):
            blk = mask[qt * QT : (qt + 1) * QT, kb * KB : (kb + 1) * KB]
            if blk.all():
                state[kb][qt] = "skip"
            elif not blk.any():
                state[kb][qt] = "full"
            else:
                tileq = (~blk.T).astype(BF)
                key = tileq.tobytes()
                if key not in pat_keys:
                    pat_keys[key] = len(patterns)
                    patterns.append(tileq)
                state[kb][qt] = pat_keys[key]
    return state, patterns


def _build_program(state, n_patterns):
    """Build the SPMD Bass program (same graph on all 8 cores)."""
    nc = bacc.Bacc(
        "TRN2", target_bir_lowering=False, debug=False, num_devices=N_CORES
    )

    p_xt = nc.dram_tensor("xt", [D_MODEL, N], BF16, kind="ExternalInput").ap()
    p_wqk = nc.dram_tensor("wqk", [D_MODEL, 1024], BF16, kind="ExternalInput").ap()
    p_wv = nc.dram_tensor("wv", [D_MODEL, 512], BF16, kind="ExternalInput").ap()
    p_wo = nc.dram_tensor("wo", [512, D_MODEL], BF16, kind="ExternalInput").ap()
    p_rope = nc.dram_tensor("rope", [128, 4, N], BF16, kind="ExternalInput").ap()
    p_wfold = nc.dram_tensor("wfold", [2, 128], BF16, kind="ExternalInput").ap()
    p_ind2 = nc.dram_tensor("ind2", [128, 2], BF16, kind="ExternalInput").ap()
    p_pswap = nc.dram_tensor("pswap", [128, 128], BF16, kind="ExternalInput").ap()
    if n_patterns:
        p_pat = nc.dram_tensor(
            "pat", [128, n_patterns, 128], BF16, kind="ExternalInput"
        ).ap()
    p_out = nc.dram_tensor("out", [N // 2, D_MODEL], F32, kind="ExternalOutput").ap()

    y_part = nc.dram_tensor("y_part", [N, D_MODEL], BF16)
    rs_out = nc.dram_tensor("rs_out", [N // 2, D_MODEL], BF16)

    QPC = TOKCH // QT  # query tiles per chunk = 4
    n_kb = [0] * NT
    qlo_t = {}
    for qc in range(NT):
        for kb in range(NKB):
            sub = [state[kb][qc * QPC + j] for j in range(QPC)]
            if all(s == "skip" for s in sub):
                continue
            n_kb[qc] = max(n_kb[qc], kb + 1)
            lead = 0
            while sub[lead] == "skip":
                lead += 1
            qlo_t[(qc, kb)] = lead

    with tile.TileContext(nc) as tc:
        import contextlib

        ctx = contextlib.ExitStack()
        with ctx:
            singles = ctx.enter_context(tc.tile_pool(name="singles", bufs=1))
            wpool = ctx.enter_context(tc.tile_pool(name="wpool", bufs=1))
            wavep = ctx.enter_context(tc.tile_pool(name="wavep", bufs=2))
            facp = ctx.enter_context(tc.tile_pool(name="facp", bufs=1))
            work = ctx.enter_context(tc.tile_pool(name="work", bufs=2))
            espool = ctx.enter_context(tc.tile_pool(name="es", bufs=3))
            epi = ctx.enter_context(tc.tile_pool(name="epi", bufs=2))
            outp = ctx.enter_context(tc.tile_pool(name="outp", bufs=2))

            pp = ctx.enter_context(tc.tile_pool(name="pp", bufs=2, space="PSUM"))
            ps = ctx.enter_context(tc.tile_pool(name="ps", bufs=2, space="PSUM"))
            pv = ctx.enter_context(tc.tile_pool(name="pv", bufs=2, space="PSUM"))
            pss = ctx.enter_context(tc.tile_pool(name="pss", bufs=2, space="PSUM"))

            # ---- resident constants -------------------------------------
            xt_sb = singles.tile([128, NDC, N], BF16)
            for dc in range(NDC):
                nc.sync.dma_start(
                    out=xt_sb[:, dc, :], in_=p_xt[dc * DC : (dc + 1) * DC, :]
                )
            rope_sb = singles.tile([128, 4, N], BF16)  # cq, sq, ck, sk
            nc.sync.dma_start(out=rope_sb, in_=p_rope)
            ident = singles.tile([128, 128], BF16)
            make_identity(nc, ident)
            ones_col = singles.tile([128, 64], BF16)
            nc.vector.memset(ones_col, 1.0)
            eps_sb = singles.tile([128, 1], F32)
            nc.vector.memset(eps_sb, EPS)
            wfold = singles.tile([2, 128], BF16)
            nc.sync.dma_start(out=wfold, in_=p_wfold)
            pswap = singles.tile([128, 128], BF16)
            nc.sync.dma_start(out=pswap, in_=p_pswap)
            ind2 = singles.tile([128, 2], BF16)
            nc.sync.dma_start(out=ind2, in_=p_ind2)
            if n_patterns:
                pat_sb = singles.tile([128, n_patterns, 128], BF16)
                nc.sync.dma_start(out=pat_sb, in_=p_pat)
            yt_sb = singles.tile([128, WAVES, N], BF16)
            wo_sb = singles.tile([128, 4, D_MODEL], BF16)
            nc.sync.dma_start(
                out=wo_sb, in_=p_wo.rearrange("(c p) f -> p c f", p=128)
            )

            for w in range(WAVES):
                # ---- wave weight slices ---------------------------------
                wq_sb = wpool.tile([128, NDC, 128], BF16, tag="wq")
                nc.sync.dma_start(
                    out=wq_sb,
                    in_=p_wqk[:, w * 128 : (w + 1) * 128].rearrange(
                        "(c p) f -> p c f", p=128
                    ),
                )
                wk_sb = wpool.tile([128, NDC, 128], BF16, tag="wk")
                nc.sync.dma_start(
                    out=wk_sb,
                    in_=p_wqk[:, 512 + w * 128 : 512 + (w + 1) * 128].rearrange(
                        "(c p) f -> p c f", p=128
                    ),
                )
                wv_sb = wpool.tile([128, NDC, 128], BF16, tag="wv")
                nc.sync.dma_start(
                    out=wv_sb,
                    in_=p_wv[:, w * 128 : (w + 1) * 128].rearrange(
                        "(c p) f -> p c f", p=128
                    ),
                )

                raw_w = wavep.tile([128, 2, N], BF16, tag="raw")  # qk raw
                ssc = facp.tile([2, 2, N], BF16, tag="ssc")  # sumsq collect
                q_rot = wavep.tile([128, N], BF16, tag="qrot")
                k_rot = wavep.tile([128, N], BF16, tag="krot")
                v_sb = wavep.tile([128, NKB, 130], BF16, tag="v")
                nc.vector.memset(v_sb[:, :, 64:65], 1.0)
                nc.vector.memset(v_sb[:, :, 129:130], 1.0)

                # ---- phase A: projections + sumsq collection ------------
                for t in range(NT):
                    tsl = slice(t * TOKCH, (t + 1) * TOKCH)
                    for qk in range(2):
                        w_sb = wq_sb if qk == 0 else wk_sb
                        pj = pp.tile([128, TOKCH], F32, tag="proj", name="pj")
                        for dc in range(NDC):
                            nc.tensor.matmul(
                                pj,
                                lhsT=w_sb[:, dc, :],
                                rhs=xt_sb[:, dc, tsl],
                                start=(dc == 0),
                                stop=(dc == NDC - 1),
                            )
                        nc.scalar.activation(raw_w[:, qk, tsl], pj, ACT.Copy)
                        sq = work.tile([128, TOKCH], BF16, tag="sq")
                        nc.scalar.activation(sq, pj, ACT.Square)
                        ssp = pss.tile([2, TOKCH], F32, tag="small", name="ssp")
                        nc.tensor.matmul(ssp, lhsT=ind2, rhs=sq, start=True, stop=True)
                        nc.scalar.activation(ssc[:, qk, tsl], ssp, ACT.Copy)
                    # V projection + transpose
                    pj = pp.tile([128, TOKCH], F32, tag="proj", name="pjv")
                    for dc in range(NDC):
                        nc.tensor.matmul(
                            pj,
                            lhsT=wv_sb[:, dc, :],
                            rhs=xt_sb[:, dc, tsl],
                            start=(dc == 0),
                            stop=(dc == NDC - 1),
                        )
                    vt = work.tile([128, TOKCH], BF16, tag="vt")
                    nc.scalar.activation(vt, pj, ACT.Copy)
                    for s in range(TOKCH // 128):
                        kb = t * (TOKCH // 128) + s
                        ptr = pp.tile([128, 128], BF16, tag="proj", name="ptr")
                        nc.tensor.transpose(ptr, vt[:, s * 128 : (s + 1) * 128], ident)
                        nc.vector.tensor_copy(v_sb[:, kb, 0:64], ptr[:, 0:64])
                        nc.vector.tensor_copy(v_sb[:, kb, 65:129], ptr[:, 64:128])

                # ---- phase B: batched RMS factors (one Ln + one Exp) ----
                lnm = facp.tile([2, 2, N], BF16, tag="lnm")
                nc.scalar.activation(
                    lnm.rearrange("p a n -> p (a n)"),
                    ssc.rearrange("p a n -> p (a n)"),
                    ACT.Ln,
                    bias=eps_sb[0:2, :],
                    scale=1.0 / D_HEAD,
                )
                inv = facp.tile([2, 2, N], BF16, tag="inv")
                nc.scalar.activation(
                    inv.rearrange("p a n -> p (a n)"),
                    lnm.rearrange("p a n -> p (a n)"),
                    ACT.Exp,
                    scale=-0.5,
                )

                # ---- phase C: normalize + rope --------------------------
                for t in range(NT):
                    tsl = slice(t * TOKCH, (t + 1) * TOKCH)
                    for qk in range(2):
                        rot = q_rot if qk == 0 else k_rot
                        fac = pss.tile([128, TOKCH], F32, tag="small", name="fac")
                        nc.tensor.matmul(
                            fac, lhsT=wfold, rhs=inv[:, qk, tsl],
                            start=True, stop=True,
                        )
                        qn = work.tile([128, TOKCH], BF16, tag="qn")
                        nc.vector.tensor_mul(qn, raw_w[:, qk, tsl], fac)
                        swp = pss.tile([128, TOKCH], F32, tag="small", name="swp")
                        nc.tensor.matmul(swp, lhsT=pswap, rhs=qn, start=True, stop=True)
                        qcos = work.tile([128, TOKCH], BF16, tag="qcos")
                        nc.vector.tensor_mul(qcos, qn, rope_sb[:, 2 * qk, tsl])
                        qsin = work.tile([128, TOKCH], BF16, tag="qsin")
                        nc.vector.tensor_mul(qsin, swp, rope_sb[:, 2 * qk + 1, tsl])
                        nc.vector.tensor_add(rot[:, tsl], qcos, qsin)

                # ---- phase D: attention ---------------------------------
                for qc in range(NT):
                    po = [
                        pv.tile([65, TOKCH], F32, tag="pv", name=f"po{h2}")
                        for h2 in range(2)
                    ]
                    first_kb = [True, True]
                    for kb in range(n_kb[qc]):
                        if (qc, kb) not in qlo_t:
                            continue
                        qlo = qlo_t[(qc, kb)] * QT
                        csl = slice(qc * TOKCH + qlo, (qc + 1) * TOKCH)
                        osl = slice(qlo, TOKCH)
                        last = kb == n_kb[qc] - 1
                        es = [None, None]
                        for h2 in range(2):
                            hr = slice(64 * h2, 64 * h2 + 64)
                            pst = ps.tile([128, TOKCH], F32, tag="s", name="pst")
                            nc.tensor.matmul(
                                pst[:, osl],
                                lhsT=k_rot[hr, kb * KB : (kb + 1) * KB],
                                rhs=q_rot[hr, csl],
                                start=True,
                                stop=True,
                            )
                            e = espool.tile([128, TOKCH], BF16, tag="es", name="es")
                            nc.scalar.activation(
                                e[:, osl], pst[:, osl], ACT.Exp,
                                scale=float(D_HEAD) ** -0.5,
                            )
                            es[h2] = e
                        for j in range(qlo // QT, QPC):
                            st = state[kb][qc * QPC + j]
                            if isinstance(st, int):
                                jsl = slice(j * QT, (j + 1) * QT)
                                for h2 in range(2):
                                    nc.vector.tensor_mul(
                                        es[h2][:, jsl], es[h2][:, jsl],
                                        pat_sb[:, st, :],
                                    )
                        for h2 in range(2):
                            nc.tensor.matmul(
                                po[h2][:, osl],
                                lhsT=v_sb[:, kb, 65 * h2 : 65 * h2 + 65],
                                rhs=es[h2][:, osl],
                                start=first_kb[h2],
                                stop=last,
                            )
                            first_kb[h2] = False
                    # epilogue: evacuate psum fast, normalize off-path
                    for h2 in range(2):
                        den = epi.tile([65, TOKCH], BF16, tag="den", name="den")
                        nc.scalar.activation(
                            den[64:65, :], po[h2][64:65, :], ACT.Copy
                        )
                        yraw = epi.tile([64, TOKCH], BF16, tag="yraw", name="yraw")
                        nc.vector.tensor_copy(yraw, po[h2][0:64, :])
                        rec = epi.tile([65, TOKCH], BF16, tag="rec", name="rec")
                        with nc.allow_low_precision(reason="softmax denom bf16"):
                            nc.vector.reciprocal(rec[64:65, :], den[64:65, :])
                        f2 = pss.tile([64, TOKCH], F32, tag="small", name="f2")
                        nc.tensor.matmul(
                            f2, lhsT=ones_col[64:65, :], rhs=rec[64:65, :],
                            start=True, stop=True,
                        )
                        nc.vector.tensor_mul(
                            yt_sb[
                                64 * h2 : 64 * h2 + 64, w,
                                qc * TOKCH : (qc + 1) * TOKCH,
                            ],
                            yraw,
                            f2,
                        )

            # ---- output projection (bf16) -------------------------------
            for t2 in range(NQT):
                for ec in range(2):
                    pot = pp.tile([128, TOKCH], F32, tag="proj", name="pot")
                    for fc in range(4):
                        nc.tensor.matmul(
                            pot,
                            lhsT=yt_sb[:, fc, t2 * 128 : (t2 + 1) * 128],
                            rhs=wo_sb[:, fc, ec * TOKCH : (ec + 1) * TOKCH],
                            start=(fc == 0),
                            stop=(fc == 3),
                        )
                    osb = outp.tile([128, TOKCH], BF16, tag="o", name="osb")
                    nc.scalar.activation(osb, pot, ACT.Copy)
                    nc.sync.dma_start(
                        out=y_part.ap()[
                            t2 * 128 : (t2 + 1) * 128,
                            ec * TOKCH : (ec + 1) * TOKCH,
                        ],
                        in_=osb,
                    )

            nc.gpsimd.collective_compute(
                "ReduceScatter",
                mybir.AluOpType.add,
                ins=[y_part.ap().opt()],
                outs=[rs_out.ap().opt()],
                replica_groups=[[0, 1], [2, 3], [4, 5], [6, 7]],
            )
            # unpack bf16 -> f32 output
            for t2 in range(NQT // 2):
                rt = outp.tile([128, D_MODEL], BF16, tag="rt", name="rt")
                nc.sync.dma_start(
                    out=rt, in_=rs_out.ap()[t2 * 128 : (t2 + 1) * 128, :]
                )
                ro = outp.tile([128, D_MODEL], F32, tag="ro", name="ro")
                nc.vector.tensor_copy(ro, rt)
                nc.sync.dma_start(
                    out=p_out[t2 * 128 : (t2 + 1) * 128, :], in_=ro
                )

    nc.compile()
    return nc


def _host_prep(x, mask, pos, W_qkv, W_out, qn_w, kn_w):
    x = np.asarray(x, dtype=np.float32)
    mask = np.asarray(mask)
    pos = np.asarray(pos).astype(np.float64)
    W_qkv = np.asarray(W_qkv, dtype=np.float32)
    W_out = np.asarray(W_out, dtype=np.float32)
    qn_w = np.asarray(qn_w, dtype=np.float32)
    kn_w = np.asarray(kn_w, dtype=np.float32)

    inv_freq = 1.0 / (ROPE_BASE ** (np.arange(0, D_HEAD, 2, dtype=np.float64) / D_HEAD))
    ang = pos[:, None] * inv_freq[None, :]  # (N, 32)
    cosT = np.cos(ang).T.astype(np.float32)  # (32, N)
    sinT = np.sin(ang).T.astype(np.float32)

    def rope_tables(w):
        cos_d = np.tile(cosT, (4, 1)) * np.tile(w, 2)[:, None]
        sin_half = np.concatenate(
            [-sinT * w[32:64][:, None], sinT * w[0:32][:, None]], axis=0
        )
        sin_d = np.tile(sin_half, (2, 1))
        return cos_d, sin_d

    cq, sq = rope_tables(qn_w)
    ck, sk = rope_tables(kn_w)
    rope = np.stack([cq, sq, ck, sk], axis=1).astype(BF)  # (128, 4, N)

    pswap_np = np.zeros((128, 128), dtype=np.float32)
    for a in range(2):
        for r in range(32):
            pswap_np[64 * a + r, 64 * a + 32 + r] = 1.0
            pswap_np[64 * a + 32 + r, 64 * a + r] = 1.0
    pswap_np = pswap_np.astype(BF)

    wfold_np = np.zeros((2, 128), dtype=np.float32)
    wfold_np[0, 0:64] = 1.0
    wfold_np[1, 64:128] = 1.0
    ind2_np = np.ascontiguousarray(wfold_np.T).astype(BF)
    wfold_np = wfold_np.astype(BF)

    state, patterns = _classify_mask(mask)
    if patterns:
        pat = np.stack(patterns, axis=1).astype(BF)
    else:
        pat = None

    q_rows = lambda h: slice(h * 192, h * 192 + 64)
    k_rows = lambda h: slice(h * 192 + 64, h * 192 + 128)
    v_rows = lambda h: slice(h * 192 + 128, h * 192 + 192)

    in_maps = []
    for c in range(N_CORES):
        b, half = divmod(c, 2)
        hs = [8 * half + i for i in range(8)]
        wqk = np.concatenate(
            [W_qkv[q_rows(h)] for h in hs] + [W_qkv[k_rows(h)] for h in hs], axis=0
        ).T
        wv = np.concatenate([W_qkv[v_rows(h)] for h in hs], axis=0).T
        wo = W_out[:, 512 * half : 512 * half + 512].T
        m = {
            "xt": np.ascontiguousarray(x[b].T).astype(BF),
            "wqk": np.ascontiguousarray(wqk).astype(BF),
            "wv": np.ascontiguousarray(wv).astype(BF),
            "wo": np.ascontiguousarray(wo).astype(BF),
            "rope": rope,
            "wfold": wfold_np,
            "ind2": ind2_np,
            "pswap": pswap_np,
        }
        if pat is not None:
            m["pat"] = pat
        in_maps.append(m)
    return in_maps, state, (0 if pat is None else pat.shape[1])


def kernel(x, mask, pos, W_qkv, W_out, qn_w, kn_w, _trace=False):
    in_maps, state, n_pat = _host_prep(x, mask, pos, W_qkv, W_out, qn_w, kn_w)
    key = (str(state), n_pat)
    if key not in _CACHE:
        _CACHE[key] = _build_program(state, n_pat)
    nc = _CACHE[key]
    res = run_bass_kernel_spmd(nc, in_maps, list(range(N_CORES)), trace=_trace)
    out = np.empty((B, N, D_MODEL), dtype=np.float32)
    for b in range(B):
        out[b, : N // 2] = res.results[2 * b]["out"]
        out[b, N // 2 :] = res.results[2 * b + 1]["out"]
    kernel._last_results = res
    return out


# revision 15
# speedup vs baseline: 1.3205x; 1.1232x over previous
"""Trainium2 Bass kernel for nn_MHA_63118839382398.

Full MHA block: fused QKV projection, per-head RMSNorm on q/k, rotate-half
RoPE, causal softmax attention, output projection.

Sharding over 8 NeuronCores: core c handles batch b = c//2 and heads
[8*(c%2), 8*(c%2)+8) (tensor parallel over head halves within a batch
pair). Each core computes a partial out-projection; a 2-rank
ReduceScatter (bf16) over each pair sums the partials and leaves each
core with half of that batch's token rows, which the host reassembles.

Layout strategy (all transposed, feats x tokens), so every matmul
contraction sits on the partition axis with no on-chip transposes except
V (cheap PE-mode 128x128 transposes):
  - xT (dmodel, ntok) per batch, host-pretransposed, bf16.
  - qT/kT = W @ xT  -> (head_dim, ntok) psum f32.
  - RMS factors: sumsq via indicator matmul, batched Ln/Exp per wave
    (one ACT table set switch pair per wave), broadcast over the 128
    rows via a tiny indicator matmul.
  - RoPE rotate-half swap via a PE permutation matmul; the qn_w/kn_w
    gains are folded into the host-built cos/sin tables.
  - scores computed transposed: S^T = K @ Q^T (keys part, queries free),
    bf16, causal blocks only; exp on ACT with the 1/sqrt(d) scale folded
    in; softmax denominator via a ones-column appended to the V
    stationary operand of the P@V matmul; 1/sum applied at the end
    (bf16 DVE reciprocal, broadcast via ones-matmul).
"""

import sys

if "/opt/trn_rl_repo" not in sys.path:
    sys.path.insert(0, "/opt/trn_rl_repo")

import numpy as np
import ml_dtypes

import concourse.bass as bass
import concourse.tile as tile
from concourse import bacc, mybir
from concourse.bass_utils import run_bass_kernel_spmd
from concourse.masks import make_identity

# Problem constants (hardcoded per harness contract).
B = 4
N = 2048
D_MODEL = 1024
N_HEADS = 16
D_HEAD = 64
ROPE_BASE = 10000.0
EPS = float(np.finfo(np.float32).eps)
N_CORES = 8

HPC = N_HEADS // 2          # heads per core = 8
WAVES = HPC // 2            # head-pair waves = 4
TOKCH = 512                 # token chunk for projections / q chunks
NT = N // TOKCH             # 4
QT = 128                    # query tile for mask classification
NQT = N // QT               # 16
KB = 128                    # key block
NKB = N // KB               # 16
DC = 128                    # dmodel chunk
NDC = D_MODEL // DC         # 8

F32 = mybir.dt.float32
BF16 = mybir.dt.bfloat16
BF = ml_dtypes.bfloat16

ACT = mybir.ActivationFunctionType

_CACHE = {}


def _pin_act_tables(arch):
    """Steer bacc's ACT-table-set choice to natural_log_exp_and_others.

    The insertion pass picks the first set containing each activation's
    function; Exp and Ln resolve to different sets by default, causing a
    ~2.7us table reload per alternation. Removing our four functions
    from every other set's *selection metadata* (runtime tables in
    act_info.json are untouched, and set ids keep their positions) makes
    all of Copy/Square/Ln/Exp resolve to the one set that has them all.
    """
    from concourse.hw_specs import get_activation_tables

    tables = get_activation_tables(arch)  # cached by reference
    keep = "natural_log_exp_and_others"
    if keep not in tables:
        return
    ours = {ACT.Copy, ACT.Square, ACT.Ln, ACT.Exp, ACT.Identity}
    for name, fns in tables.items():
        if name != keep:
            fns -= ours


def _classify_mask(mask):
    """Per (key-block, query-tile) classification of the mask.

    Returns (state[NKB][NQT], patterns): state is 'skip' (all masked),
    'full' (none masked), or an index into patterns: unique (128,128)
    bf16 0/1 tiles indexed [key, query]."""
    mask = np.asarray(mask)
    assert mask.shape == (N, N)
    patterns = []
    pat_keys = {}
    state = [[None] * NQT for _ in range(NKB)]
    for kb in range(NKB):
        for qt in range(NQT):
            blk = mask[qt * QT : (qt + 1) * QT, kb * KB : (kb + 1) * KB]
            if blk.all():
                state[kb][qt] = "skip"
            elif not blk.any():
                state[kb][qt] = "full"
            else:
                tileq = (~blk.T).astype(BF)
                key = tileq.tobytes()
                if key not in pat_keys:
                    pat_keys[key] = len(patterns)
                    patterns.append(tileq)
                state[kb][qt] = pat_keys[key]
    return state, patterns


def _build_program(state, n_patterns):
    """Build the SPMD Bass program (same graph on all 8 cores)."""
    nc = bacc.Bacc(
        "TRN2", target_bir_lowering=False, debug=False, num_devices=N_CORES
    )
    _pin_act_tables(nc.m.arch)

    p_xt = nc.dram_tensor("xt", [D_MODEL, N], BF16, kind="ExternalInput").ap()
    p_wqk = nc.dram_tensor("wqk", [D_MODEL, 1024], BF16, kind="ExternalInput").ap()
    p_wv = nc.dram_tensor("wv", [D_MODEL, 512], BF16, kind="ExternalInput").ap()
    p_wo = nc.dram_tensor("wo", [512, D_MODEL], BF16, kind="ExternalInput").ap()
    p_rope = nc.dram_tensor("rope", [128, 4, N], BF16, kind="ExternalInput").ap()
    p_wfold = nc.dram_tensor("wfold", [2, 128], BF16, kind="ExternalInput").ap()
    p_ind2 = nc.dram_tensor("ind2", [128, 2], BF16, kind="ExternalInput").ap()
    p_pswap = nc.dram_tensor("pswap", [128, 128], BF16, kind="ExternalInput").ap()
    if n_patterns:
        p_pat = nc.dram_tensor(
            "pat", [128, n_patterns, 128], BF16, kind="ExternalInput"
        ).ap()
    p_out = nc.dram_tensor("out", [N // 2, D_MODEL], F32, kind="ExternalOutput").ap()

    y_part = nc.dram_tensor("y_part", [N, D_MODEL], BF16)
    rs_out = nc.dram_tensor("rs_out", [N // 2, D_MODEL], BF16)

    QPC = TOKCH // QT  # query tiles per chunk = 4
    n_kb = [0] * NT
    qlo_t = {}
    for qc in range(NT):
        for kb in range(NKB):
            sub = [state[kb][qc * QPC + j] for j in range(QPC)]
            if all(s == "skip" for s in sub):
                continue
            n_kb[qc] = max(n_kb[qc], kb + 1)
            lead = 0
            while sub[lead] == "skip":
                lead += 1
            qlo_t[(qc, kb)] = lead

    with tile.TileContext(nc) as tc:
        import contextlib

        ctx = contextlib.ExitStack()
        with ctx:
            singles = ctx.enter_context(tc.tile_pool(name="singles", bufs=1))
            wpool = ctx.enter_context(tc.tile_pool(name="wpool", bufs=1))
            wavep = ctx.enter_context(tc.tile_pool(name="wavep", bufs=2))
            facp = ctx.enter_context(tc.tile_pool(name="facp", bufs=2))
            work = ctx.enter_context(tc.tile_pool(name="work", bufs=2))
            espool = ctx.enter_context(tc.tile_pool(name="es", bufs=3))
            epi = ctx.enter_context(tc.tile_pool(name="epi", bufs=2))
            outp = ctx.enter_context(tc.tile_pool(name="outp", bufs=2))

            pp = ctx.enter_context(tc.tile_pool(name="pp", bufs=2, space="PSUM"))
            ps = ctx.enter_context(tc.tile_pool(name="ps", bufs=2, space="PSUM"))
            pv = ctx.enter_context(tc.tile_pool(name="pv", bufs=2, space="PSUM"))
            pss = ctx.enter_context(tc.tile_pool(name="pss", bufs=2, space="PSUM"))

            # ---- resident constants -------------------------------------
            xt_sb = singles.tile([128, NDC, N], BF16)
            for dc in range(NDC):
                nc.sync.dma_start(
                    out=xt_sb[:, dc, :], in_=p_xt[dc * DC : (dc + 1) * DC, :]
                )
            rope_sb = singles.tile([128, 4, N], BF16)  # cq, sq, ck, sk
            ident = singles.tile([128, 128], BF16)
            make_identity(nc, ident)
            ones_col = singles.tile([128, 64], BF16)
            nc.vector.memset(ones_col, 1.0)
            eps_sb = singles.tile([128, 1], F32)
            nc.vector.memset(eps_sb, EPS)
            wfold = singles.tile([2, 128], BF16)
            nc.sync.dma_start(out=wfold, in_=p_wfold)
            pswap = singles.tile([128, 128], BF16)
            nc.sync.dma_start(out=pswap, in_=p_pswap)
            ind2 = singles.tile([128, 2], BF16)
            nc.sync.dma_start(out=ind2, in_=p_ind2)
            if n_patterns:
                pat_sb = singles.tile([128, n_patterns, 128], BF16)
            yt_sb = singles.tile([128, WAVES, N], BF16)
            wo_sb = singles.tile([128, 4, D_MODEL], BF16)

            def emit_A(w):
                """Projections + sumsq for wave w (head pair 2w, 2w+1)."""
                t_ = {}
                wq_sb = wpool.tile([128, NDC, 128], BF16, tag="wq", name="wq_sb")
                nc.sync.dma_start(
                    out=wq_sb,
                    in_=p_wqk[:, w * 128 : (w + 1) * 128].rearrange(
                        "(c p) f -> p c f", p=128
                    ),
                )
                wk_sb = wpool.tile([128, NDC, 128], BF16, tag="wk", name="wk_sb")
                nc.sync.dma_start(
                    out=wk_sb,
                    in_=p_wqk[:, 512 + w * 128 : 512 + (w + 1) * 128].rearrange(
                        "(c p) f -> p c f", p=128
                    ),
                )
                wv_sb = wpool.tile([128, NDC, 128], BF16, tag="wv", name="wv_sb")
                nc.sync.dma_start(
                    out=wv_sb,
                    in_=p_wv[:, w * 128 : (w + 1) * 128].rearrange(
                        "(c p) f -> p c f", p=128
                    ),
                )
                raw_w = wavep.tile([128, 2, N], BF16, tag="raw", name="raw_w")
                q_rot = wavep.tile([128, N], BF16, tag="qrot", name="q_rot")
                k_rot = wavep.tile([128, N], BF16, tag="krot", name="k_rot")
                v_sb = wavep.tile([128, NKB, 130], BF16, tag="v", name="v_sb")
                nc.vector.memset(v_sb[:, :, 64:65], 1.0)
                nc.vector.memset(v_sb[:, :, 129:130], 1.0)
                inv_w = facp.tile([2, 2, N], BF16, tag="inv", name="inv_w")

                for t in range(NT):
                    tsl = slice(t * TOKCH, (t + 1) * TOKCH)
                    for qk in range(2):
                        w_sb = wq_sb if qk == 0 else wk_sb
                        pj = pp.tile([128, TOKCH], F32, tag="proj", name="pj")
                        for dc in range(NDC):
                            nc.tensor.matmul(
                                pj,
                                lhsT=w_sb[:, dc, :],
                                rhs=xt_sb[:, dc, tsl],
                                start=(dc == 0),
                                stop=(dc == NDC - 1),
                            )
                        nc.scalar.activation(raw_w[:, qk, tsl], pj, ACT.Copy)
                        sq = work.tile([128, TOKCH], BF16, tag="sq")
                        nc.vector.tensor_mul(sq, raw_w[:, qk, tsl], raw_w[:, qk, tsl])
                        ssp = pss.tile([2, TOKCH], F32, tag="small", name="ssp")
                        nc.tensor.matmul(ssp, lhsT=ind2, rhs=sq, start=True, stop=True)
                        lnm = work.tile([2, TOKCH], F32, tag="lnm")
                        nc.scalar.activation(
                            lnm, ssp, ACT.Ln, bias=eps_sb[0:2, :], scale=1.0 / D_HEAD
                        )
                        nc.scalar.activation(
                            inv_w[:, qk, tsl], lnm, ACT.Exp, scale=-0.5
                        )
                    # V projection + transpose
                    pj = pp.tile([128, TOKCH], F32, tag="proj", name="pjv")
                    for dc in range(NDC):
                        nc.tensor.matmul(
                            pj,
                            lhsT=wv_sb[:, dc, :],
                            rhs=xt_sb[:, dc, tsl],
                            start=(dc == 0),
                            stop=(dc == NDC - 1),
                        )
                    vt = work.tile([128, TOKCH], BF16, tag="vt")
                    nc.vector.tensor_copy(vt, pj)
                    for sview in range(TOKCH // 128):
                        kb = t * (TOKCH // 128) + sview
                        ptr = pp.tile([128, 128], BF16, tag="proj", name="ptr")
                        nc.tensor.transpose(
                            ptr, vt[:, sview * 128 : (sview + 1) * 128], ident
                        )
                        nc.vector.tensor_copy(v_sb[:, kb, 0:64], ptr[:, 0:64])
                        nc.vector.tensor_copy(v_sb[:, kb, 65:129], ptr[:, 64:128])
                t_.update(raw_w=raw_w, q_rot=q_rot, k_rot=k_rot, v_sb=v_sb, inv_w=inv_w)
                return t_

            def emit_BCD(w, t_):
                raw_w, q_rot, k_rot, v_sb = (
                    t_["raw_w"], t_["q_rot"], t_["k_rot"], t_["v_sb"]
                )
                inv_w = t_["inv_w"]
                # ---- normalize + rope -----------------------------------
                for t in range(NT):
                    tsl = slice(t * TOKCH, (t + 1) * TOKCH)
                    for qk in range(2):
                        rot = q_rot if qk == 0 else k_rot
                        fac = pss.tile([128, TOKCH], F32, tag="small", name="fac")
                        nc.tensor.matmul(
                            fac, lhsT=wfold, rhs=inv_w[:, qk, tsl],
                            start=True, stop=True,
                        )
                        qn = work.tile([128, TOKCH], BF16, tag="qn")
                        nc.vector.tensor_mul(qn, raw_w[:, qk, tsl], fac)
                        swp = pss.tile([128, TOKCH], F32, tag="small", name="swp")
                        nc.tensor.matmul(
                            swp, lhsT=pswap, rhs=qn, start=True, stop=True
                        )
                        qcos = work.tile([128, TOKCH], BF16, tag="qcos")
                        nc.vector.tensor_mul(qcos, qn, rope_sb[:, 2 * qk, tsl])
                        qsin = work.tile([128, TOKCH], BF16, tag="qsin")
                        nc.vector.tensor_mul(qsin, swp, rope_sb[:, 2 * qk + 1, tsl])
                        nc.vector.tensor_add(rot[:, tsl], qcos, qsin)

                # ---- attention ------------------------------------------
                for qc in range(NT):
                    po = [
                        pv.tile([65, TOKCH], F32, tag="pv", name=f"po{h2}")
                        for h2 in range(2)
                    ]
                    first_kb = [True, True]
                    for kb in range(n_kb[qc]):
                        if (qc, kb) not in qlo_t:
                            continue
                        qlo = qlo_t[(qc, kb)] * QT
                        csl = slice(qc * TOKCH + qlo, (qc + 1) * TOKCH)
                        osl = slice(qlo, TOKCH)
                        last = kb == n_kb[qc] - 1
                        es = [None, None]
                        for h2 in range(2):
                            hr = slice(64 * h2, 64 * h2 + 64)
                            pst = ps.tile([128, TOKCH], F32, tag="s", name="pst")
                            nc.tensor.matmul(
                                pst[:, osl],
                                lhsT=k_rot[hr, kb * KB : (kb + 1) * KB],
                                rhs=q_rot[hr, csl],
                                start=True,
                                stop=True,
                            )
                            e = espool.tile([128, TOKCH], BF16, tag="es", name="es")
                            nc.scalar.activation(
                                e[:, osl], pst[:, osl], ACT.Exp,
                                scale=float(D_HEAD) ** -0.5,
                            )
                            es[h2] = e
                        for j in range(qlo // QT, QPC):
                            st = state[kb][qc * QPC + j]
                            if isinstance(st, int):
                                jsl = slice(j * QT, (j + 1) * QT)
                                for h2 in range(2):
                                    nc.vector.tensor_mul(
                                        es[h2][:, jsl], es[h2][:, jsl],
                                        pat_sb[:, st, :],
                                    )
                        for h2 in range(2):
                            nc.tensor.matmul(
                                po[h2][:, osl],
                                lhsT=v_sb[:, kb, 65 * h2 : 65 * h2 + 65],
                                rhs=es[h2][:, osl],
                                start=first_kb[h2],
                                stop=last,
                            )
                            first_kb[h2] = False
                    # epilogue: rec = exp(-ln(denom)); yt = yraw * bcast(rec)
                    for h2 in range(2):
                        lnd = epi.tile([65, TOKCH], F32, tag="lnd", name="lnd")
                        nc.scalar.activation(
                            lnd[64:65, :], po[h2][64:65, :], ACT.Ln
                        )
                        yraw = epi.tile([64, TOKCH], BF16, tag="yraw", name="yraw")
                        nc.vector.tensor_copy(yraw, po[h2][0:64, :])
                        rec = epi.tile([65, TOKCH], BF16, tag="rec", name="rec")
                        nc.scalar.activation(
                            rec[64:65, :], lnd[64:65, :], ACT.Exp, scale=-1.0
                        )
                        f2 = pss.tile([64, TOKCH], F32, tag="small", name="f2")
                        nc.tensor.matmul(
                            f2, lhsT=ones_col[64:65, :], rhs=rec[64:65, :],
                            start=True, stop=True,
                        )
                        nc.vector.tensor_mul(
                            yt_sb[
                                64 * h2 : 64 * h2 + 64, w,
                                qc * TOKCH : (qc + 1) * TOKCH,
                            ],
                            yraw,
                            f2,
                        )

            # software-pipelined wave emission: A(w+1) ahead of BCD(w)
            wave_tiles = {}
            for seg in range(WAVES + 1):
                if seg < WAVES:
                    wave_tiles[seg] = emit_A(seg)
                if seg == 0:
                    # deferred bulk constants (behind wave-0 weights in queue)
                    nc.sync.dma_start(out=rope_sb, in_=p_rope)
                    if n_patterns:
                        nc.sync.dma_start(out=pat_sb, in_=p_pat)
                    nc.sync.dma_start(
                        out=wo_sb, in_=p_wo.rearrange("(c p) f -> p c f", p=128)
                    )
                if seg >= 1:
                    emit_BCD(seg - 1, wave_tiles.pop(seg - 1))

            # ---- output projection (bf16) -------------------------------
            for t2 in range(NQT):
                for ec in range(2):
                    pot = pp.tile([128, TOKCH], F32, tag="proj", name="pot")
                    for fc in range(4):
                        nc.tensor.matmul(
                            pot,
                            lhsT=yt_sb[:, fc, t2 * 128 : (t2 + 1) * 128],
                            rhs=wo_sb[:, fc, ec * TOKCH : (ec + 1) * TOKCH],
                            start=(fc == 0),
                            stop=(fc == 3),
                        )
                    osb = outp.tile([128, TOKCH], BF16, tag="o", name="osb")
                    nc.scalar.activation(osb, pot, ACT.Copy)
                    nc.sync.dma_start(
                        out=y_part.ap()[
                            t2 * 128 : (t2 + 1) * 128,
                            ec * TOKCH : (ec + 1) * TOKCH,
                        ],
                        in_=osb,
                    )

            nc.gpsimd.collective_compute(
                "ReduceScatter",
                mybir.AluOpType.add,
                ins=[y_part.ap().opt()],
                outs=[rs_out.ap().opt()],
                replica_groups=[[0, 1], [2, 3], [4, 5], [6, 7]],
            )
            # unpack bf16 -> f32 output
            for t2 in range(NQT // 2):
                rt = outp.tile([128, D_MODEL], BF16, tag="rt", name="rt")
                nc.sync.dma_start(
                    out=rt, in_=rs_out.ap()[t2 * 128 : (t2 + 1) * 128, :]
                )
                ro = outp.tile([128, D_MODEL], F32, tag="ro", name="ro")
                nc.vector.tensor_copy(ro, rt)
                nc.sync.dma_start(
                    out=p_out[t2 * 128 : (t2 + 1) * 128, :], in_=ro
                )

    nc.compile()
    return nc


def _host_prep(x, mask, pos, W_qkv, W_out, qn_w, kn_w):
    x = np.asarray(x, dtype=np.float32)
    mask = np.asarray(mask)
    pos = np.asarray(pos).astype(np.float64)
    W_qkv = np.asarray(W_qkv, dtype=np.float32)
    W_out = np.asarray(W_out, dtype=np.float32)
    qn_w = np.asarray(qn_w, dtype=np.float32)
    kn_w = np.asarray(kn_w, dtype=np.float32)

    inv_freq = 1.0 / (ROPE_BASE ** (np.arange(0, D_HEAD, 2, dtype=np.float64) / D_HEAD))
    ang = pos[:, None] * inv_freq[None, :]  # (N, 32)
    cosT = np.cos(ang).T.astype(np.float32)  # (32, N)
    sinT = np.sin(ang).T.astype(np.float32)

    def rope_tables(w):
        cos_d = np.tile(cosT, (4, 1)) * np.tile(w, 2)[:, None]
        sin_half = np.concatenate(
            [-sinT * w[32:64][:, None], sinT * w[0:32][:, None]], axis=0
        )
        sin_d = np.tile(sin_half, (2, 1))
        return cos_d, sin_d

    cq, sq = rope_tables(qn_w)
    ck, sk = rope_tables(kn_w)
    rope = np.stack([cq, sq, ck, sk], axis=1).astype(BF)  # (128, 4, N)

    pswap_np = np.zeros((128, 128), dtype=np.float32)
    for a in range(2):
        for r in range(32):
            pswap_np[64 * a + r, 64 * a + 32 + r] = 1.0
            pswap_np[64 * a + 32 + r, 64 * a + r] = 1.0
    pswap_np = pswap_np.astype(BF)

    wfold_np = np.zeros((2, 128), dtype=np.float32)
    wfold_np[0, 0:64] = 1.0
    wfold_np[1, 64:128] = 1.0
    ind2_np = np.ascontiguousarray(wfold_np.T).astype(BF)
    wfold_np = wfold_np.astype(BF)

    state, patterns = _classify_mask(mask)
    if patterns:
        pat = np.stack(patterns, axis=1).astype(BF)
    else:
        pat = None

    q_rows = lambda h: slice(h * 192, h * 192 + 64)
    k_rows = lambda h: slice(h * 192 + 64, h * 192 + 128)
    v_rows = lambda h: slice(h * 192 + 128, h * 192 + 192)

    in_maps = []
    for c in range(N_CORES):
        b, half = divmod(c, 2)
        hs = [8 * half + i for i in range(8)]
        wqk = np.concatenate(
            [W_qkv[q_rows(h)] for h in hs] + [W_qkv[k_rows(h)] for h in hs], axis=0
        ).T
        wv = np.concatenate([W_qkv[v_rows(h)] for h in hs], axis=0).T
        wo = W_out[:, 512 * half : 512 * half + 512].T
        m = {
            "xt": np.ascontiguousarray(x[b].T).astype(BF),
            "wqk": np.ascontiguousarray(wqk).astype(BF),
            "wv": np.ascontiguousarray(wv).astype(BF),
            "wo": np.ascontiguousarray(wo).astype(BF),
            "rope": rope,
            "wfold": wfold_np,
            "ind2": ind2_np,
            "pswap": pswap_np,
        }
        if pat is not None:
            m["pat"] = pat
        in_maps.append(m)
    return in_maps, state, (0 if pat is None else pat.shape[1])


def kernel(x, mask, pos, W_qkv, W_out, qn_w, kn_w, _trace=False):
    in_maps, state, n_pat = _host_prep(x, mask, pos, W_qkv, W_out, qn_w, kn_w)
    key = (str(state), n_pat)
    if key not in _CACHE:
        _CACHE[key] = _build_program(state, n_pat)
    nc = _CACHE[key]
    res = run_bass_kernel_spmd(nc, in_maps, list(range(N_CORES)), trace=_trace)
    out = np.empty((B, N, D_MODEL), dtype=np.float32)
    for b in range(B):
        out[b, : N // 2] = res.results[2 * b]["out"]
        out[b, N // 2 :] = res.results[2 * b + 1]["out"]
    kernel._last_results = res
    return out
